# revision 70
# baseline (speedup 1.0000x reference)
"""Trainium2 Bass kernel for nn_BBoxDecoder (HyperNetwork -> per-sample CoordinateNet).

Computation:
    h1   = relu(z @ W1.T + b1)            (32, 512)
    h2   = relu(h1 @ W2.T + b2)           (32, 1024)
    flat = h2 @ W3.T + b3                 (32, 198916)   <- W3 is the bottleneck
    per-sample 5-layer CoordinateNet on timestamps -> (32, 512, 4)

Distribution over 8 NeuronCores:
  - W3 is sharded along the output-param dim 8 ways; each core streams its
    shard ONCE as a single fp16 plane (~51.4 MB, the memory roofline) and
    computes flat[:, shard] in one fp16 PE pass per tile (vs the previous
    fp16 hi/lo pair: half the HBM traffic and 1/3 the PE passes).
  - flat shards are exchanged with 4 pipelined AllToAll collectives whose
    group boundaries are aligned to CoordinateNet layer needs, so each layer
    starts as soon as its params arrived (overlapping the W3 stream).
  - The CoordinateNet application is data-parallel over the batch (4/core)
    and is emitted gated on the final drain so it never stalls the stream.

Engine-queue discipline (in-order queues; a blocked wait at a queue head
stalls everything behind it, so each queue carries ONE kind of work):
  SP    = W3/b3/const stream DMAs + a2a_in writes (stream-ordered)
  PE    = stream matmuls, then the end-staged CoordinateNet
  DVE   = PSUM->flat drains (fp16)
  ACT   = activations + mid-stream extract DMAs (f4g/pieces for g0-g2)
  Pool  = the 4 collectives + the last group's extracts (idle post-coll)

Precision plan (hardware-validated: rel 1.059e-2 < 2e-2 gate):
  z/W1 fp16 -> h1 fp32; W2 fp32 (fp32 matmul); h2 fp16; W3+b3 fp16 single
  plane; flat + exchange fp16; CoordinateNet params/x/timestamps fp16 with
  fp32 PSUM accumulation; biases applied fp32 via ACT.

Modeled HW exec: 221,324 ns (baseline hi/lo kernel: 447,868 ns -> 2.02x).
"""

import os
import sys

import numpy as np

if os.path.isdir("/opt/trn_rl_repo") and "/opt/trn_rl_repo" not in sys.path:
    sys.path.insert(0, "/opt/trn_rl_repo")

import concourse.bass as bass
import concourse.mybir as mybir
import concourse.tile as tile
from concourse.bass import ts
from concourse.bass_utils import run_bass_kernel_spmd

# ---------------------------------------------------------------- constants
B = 32          # batch
NPTS = 512      # timestamps per sample
LAT = 4096      # latent dim
H1 = 512        # hyper hidden 1
H2 = 1024       # hyper hidden 2
HID = 256       # CoordinateNet hidden dim
P_TOTAL = 198916

NCORES = 8
CH = 512                  # matmul free-dim chunk
BPC = B // NCORES         # 4 samples per core
S = 24896                 # per-core shard of the (padded) param dim
P_PAD = NCORES * S        # 199168 (pad only 252 params past P_TOTAL)

# Exchange groups (per-core widths). Boundaries are aligned to CoordinateNet
# layer needs AND to stream-subgroup edges so each AllToAll's input is drained
# just before it fires, with arrival spacing >= collective service time:
#   g0 ends at 8*8320  = 66560  >= 66304  (end of Wh0+bh0) -> input+hidden0
#   g1 ends at 8*16512 = 132096 == end of Wh1+bh1          -> hidden1
#   g2..g4 cover Wh2 + Wo/bo                               -> hidden2+output
EW = [8320, 8192, 4480, 3904]
NG = len(EW)
ECUM = [0]
for _w in EW:
    ECUM.append(ECUM[-1] + _w)
assert ECUM[-1] == S
GSTART = [NCORES * c for c in ECUM[:-1]]   # global start of group g
GBS = [NCORES * w for w in EW]             # global params per group

# Stream subgroups: boundaries END exactly at each exchange group's gating
# column (8704 for g0, 16896 for g1, 20992 for g2), so an AllToAll's input
# drains as early as possible -- the collective chain is device-serial from
# g1 onward, and every us earlier here moves the whole tail.
SSG = [3072, 3072, 2560, 3072, 3072, 2048, 2048, 2048, 3072, 832]
SCUM = [0]
for _w in SSG:
    SCUM.append(SCUM[-1] + _w)
assert SCUM[-1] == S

WH_OFF = [512 + l * (HID * HID + HID) for l in range(3)]  # 512, 66304, 132096
WO_OFF = 197888
BO_OFF = 198912

FP = mybir.dt.float32
F16 = mybir.dt.float16
AF = mybir.ActivationFunctionType


# ------------------------------------------------------------- wait splitter
def _split_multi_waits(nc):
    """The walrus build here accepts at most one sync-wait per instruction.
    Engines execute in order, so hoisting all but the last wait onto fresh
    NOPs immediately before the instruction is semantically identical."""
    ctr = 0
    for f in nc.m.functions:
        for bb in f.blocks:
            out = []
            changed = False
            for ins in bb.instructions:
                si = getattr(ins, "sync_info", None)
                waits = list(si.on_wait) if (si is not None and si.on_wait) else []
                if len(waits) > 1:
                    changed = True
                    for w in waits[:-1]:
                        ctr += 1
                        out.append(
                            mybir.InstNoOp(
                                name=f"{ins.name}-sw{ctr}",
                                engine=ins.engine,
                                sync_info=mybir.SyncInfo(on_wait=[w], on_update=[]),
                            )
                        )
                    ins.sync_info = mybir.SyncInfo(
                        on_wait=waits[-1:], on_update=list(si.on_update or [])
                    )
                out.append(ins)
            if changed:
                try:
                    bb.instructions = out
                except Exception:
                    bb.instructions.clear()
                    bb.instructions.extend(out)


# ------------------------------------------------------------ device program
def _build_module(repeat: int = 1):
    nc = bass.Bass(num_devices=NCORES)

    zt_d = nc.dram_tensor("zt", [128, LAT // 128, B], F16, kind="ExternalInput")
    w1t_d = nc.dram_tensor("w1t", [LAT, H1], F16, kind="ExternalInput")
    b1_d = nc.dram_tensor("b1", [H1], FP, kind="ExternalInput")
    w2t_d = nc.dram_tensor("w2t", [H1, H2], FP, kind="ExternalInput")
    b2_d = nc.dram_tensor("b2", [H2], FP, kind="ExternalInput")
    w3_d = nc.dram_tensor("w3", [H2 + 1, S], F16, kind="ExternalInput")
    ts_d = nc.dram_tensor("tst", [BPC, NPTS], F16, kind="ExternalInput")
    # out[p, m, j, o] <-> out[j, m*128+p, o]; host does the final transpose
    out_d = nc.dram_tensor("out", [128, BPC, 4, 4], FP, kind="ExternalOutput")

    with tile.TileContext(nc) as tc:
        with (
            tc.tile_pool(name="const", bufs=1) as const,
            tc.tile_pool(name="w1p", bufs=4) as w1p,
            tc.tile_pool(name="w3p", bufs=4) as w3p,
            tc.tile_pool(name="b3p", bufs=3) as b3p,
            tc.tile_pool(name="cpool", bufs=1) as cpool,
            tc.tile_pool(name="xpool", bufs=8) as xpool,
            tc.tile_pool(name="opool", bufs=4) as opool,
            tc.tile_pool(name="psum", bufs=8, space="PSUM") as psum,
            tc.tile_pool(name="dram", bufs=1, space="DRAM") as dram,
        ):
            for _rep in range(repeat):
                _emit_body(nc, tc, const, w1p, w3p, b3p, cpool, xpool, opool,
                           psum, dram, zt_d, w1t_d, b1_d, w2t_d, b2_d,
                           w3_d, ts_d, out_d)

    _split_multi_waits(nc)
    return nc


def _emit_body(nc, tc, const, w1p, w3p, b3p, cpool, xpool, opool, psum,
               dram, zt_d, w1t_d, b1_d, w2t_d, b2_d, w3_d, ts_d, out_d):
    # ---- constant loads (order = DMA queue order: critical path first)
    zsb = const.tile([128, LAT // 128, B], F16, name="zsb", tag="zsb")
    nc.sync.dma_start(zsb[:], zt_d[:, :, :].rearrange("p t b -> p t b"))
    b1sb = const.tile([128, H1 // 128], FP, name="b1sb", tag="b1sb")
    nc.sync.dma_start(b1sb[:], b1_d[:].rearrange("(t p) -> p t", p=128))

    w1sbs = []
    for kk in range(8):
        w1sb = w1p.tile([128, 4, H1], F16, name="w1sb", tag="w1sb")
        nc.sync.dma_start(
            w1sb[:],
            w1t_d[ts(kk, 512), :].rearrange("(t p) m -> p t m", p=128),
        )
        w1sbs.append(w1sb)

    w2sb = const.tile([128, H1 // 128, H2], FP, name="w2sb", tag="w2sb")
    nc.sync.dma_start(w2sb[:], w2t_d[:, :].rearrange("(t p) m -> p t m", p=128))
    b2sb = const.tile([128, H2 // 128], FP, name="b2sb", tag="b2sb")
    nc.sync.dma_start(b2sb[:], b2_d[:].rearrange("(t p) -> p t", p=128))
    tssb = const.tile([1, BPC, NPTS], F16, name="tssb", tag="tssb")
    nc.sync.dma_start(tssb[:], ts_d[:, :].rearrange("(a j) n -> a j n", a=1))
    tss2 = const.tile([1, BPC, NPTS], F16, name="tss2", tag="tss2")
    ones16 = const.tile([1, B], F16, name="ones16", tag="ones16")
    nc.gpsimd.memset(ones16[:], 1.0)
    ones128 = const.tile([1, 128], F16, name="ones128", tag="ones128")
    nc.gpsimd.memset(ones128[:], 1.0)

    # ---- h1T = relu(W1 @ z.T + b1), stored (512, 32) as [128, 4, 32] fp32
    h1sb = const.tile([128, 4, B], FP, name="h1sb", tag="h1sb")
    h1ps = [psum.tile([128, B], FP, name=f"h1ps{m}", tag="ps") for m in range(4)]
    for kk in range(8):
        for t4 in range(4):
            k = kk * 4 + t4
            for m in range(4):
                nc.tensor.matmul(
                    h1ps[m][:],
                    w1sbs[kk][:, t4, ts(m, 128)],
                    zsb[:, k, :],
                    start=(k == 0),
                    stop=(k == 31),
                )
    for m in range(4):
        nc.scalar.activation(
            h1sb[:, m, :], h1ps[m][:], AF.Relu, bias=b1sb[:, m : m + 1]
        )

    # ---- h2 = relu(W2 @ h1 + b2) as fp16 [128, 8, 32] (fp32 matmul)
    h2h = const.tile([128, 8, B], F16, name="h2h", tag="h2h")
    for m in range(8):
        h2ps = psum.tile([128, B], FP, name="h2ps", tag="ps")
        for k in range(4):
            nc.tensor.matmul(
                h2ps[:],
                w2sb[:, k, ts(m, 128)],
                h1sb[:, k, :],
                start=(k == 0),
                stop=(k == 3),
            )
        nc.scalar.activation(
            h2h[:, m, :], h2ps[:], AF.Relu, bias=b2sb[:, m : m + 1]
        )

    # ---- CoordinateNet param tiles (fp16), batched over the 4 samples
    win4 = cpool.tile([1, BPC, HID], F16, name="win4", tag="win4")
    bin4 = cpool.tile([128, BPC, 2], F16, name="bin4", tag="bin4")
    binf = cpool.tile([128, BPC, 2], FP, name="binf", tag="binf")
    wh4s = []
    bh4s = []
    bhfs = []
    for l in range(3):
        wh4s.append(cpool.tile([128, BPC, 2, HID], F16, name=f"wh4_{l}", tag=f"wh4_{l}"))
        bh4s.append(cpool.tile([128, BPC, 2], F16, name=f"bh4_{l}", tag=f"bh4_{l}"))
        bhfs.append(cpool.tile([128, BPC, 2], FP, name=f"bhf_{l}", tag=f"bhf_{l}"))
    wo4 = cpool.tile([128, BPC, 2, 4], F16, name="wo4", tag="wo4")
    bo4 = cpool.tile([1, BPC, 4], F16, name="bo4", tag="bo4")

    def _extract_pieces(g, f4g, eng, eng2=None):
        eng2 = eng2 or eng
        """Emit DMAs for every param piece inside exchange group g.
        dst[p, j, t(, o)] <- f4g[j, ...]; boundaries are inner-aligned."""
        blocks = [(win4, 0, HID, HID, True)]
        blocks.append((bin4, HID, HID, 1, False))
        for l in range(3):
            a = WH_OFF[l]
            blocks.append((wh4s[l], a, HID * HID, HID, False))
            blocks.append((bh4s[l], a + HID * HID, HID, 1, False))
        blocks.append((wo4, WO_OFF, 4 * HID, 4, False))
        blocks.append((bo4, BO_OFF, 4, 4, True))
        for dst_tile, a, length, inner, single_row in blocks:
            lo = max(a, GSTART[g])
            hi = min(a + length, GSTART[g] + GBS[g])
            if lo >= hi:
                continue
            if single_row:
                off = lo - GSTART[g]
                src = f4g[:, off : off + (hi - lo)].rearrange(
                    "(a j) o -> a j o", a=1
                )
                eng2.dma_start(dst_tile[0:1, :, lo - a : hi - a], src)
                continue
            i0 = (lo - a) // inner
            i1 = (hi - a) // inner
            for t in range(2):
                pa = max(i0, 128 * t)
                pb = min(i1, 128 * (t + 1))
                if pa >= pb:
                    continue
                gl = a + pa * inner - GSTART[g]
                src = f4g[:, gl : gl + (pb - pa) * inner].rearrange(
                    "j (p o) -> p j o", o=inner
                )
                if inner == 1:
                    dst = dst_tile[pa - 128 * t : pb - 128 * t, :, t : t + 1]
                    eng2.dma_start(dst, src)
                else:
                    dst = dst_tile[pa - 128 * t : pb - 128 * t, :, t, :]
                    (eng if inner == HID else eng2).dma_start(dst, src)

    xs = [None] * BPC

    def _input_layer():
        for j in range(BPC):
            xc = xpool.tile([128, 2, NPTS], F16, name="xt", tag="xt")
            for t in range(2):
                xps = psum.tile([128, NPTS], FP, name="xps", tag="ps")
                nc.tensor.matmul(
                    xps[:], win4[0:1, j, ts(t, 128)], tss2[0:1, j, :],
                    start=True, stop=True,
                )
                nc.scalar.activation(
                    xc[:, t, :], xps[:], AF.Relu, bias=binf[:, j, t : t + 1]
                )
            xs[j] = xc

    def _hidden_layer(l):
        for j in range(BPC):
            xn = xpool.tile([128, 2, NPTS], F16, name="xt", tag="xt")
            for m in range(2):
                hps = psum.tile([128, NPTS], FP, name="hps", tag="ps")
                for t in range(2):
                    nc.tensor.matmul(
                        hps[:], wh4s[l][:, j, t, ts(m, 128)], xs[j][:, t, :],
                        start=(t == 0), stop=(t == 1),
                    )
                nc.scalar.activation(
                    xn[:, m, :], hps[:], AF.Relu, bias=bhfs[l][:, j, m : m + 1]
                )
            xs[j] = xn

    def _output_layer():
        out_sb = opool.tile([128, BPC, 4, 4], FP, name="out_sb", tag="out_sb")
        ops_ = psum.tile([128, BPC, 4, 4], FP, name="ops", tag="ps")
        for j in range(BPC):
            for m in range(4):
                for t in range(2):
                    nc.tensor.matmul(
                        ops_[:, j, m, :], xs[j][:, t, ts(m, 128)],
                        wo4[:, j, t, :], start=(t == 0), stop=False,
                    )
                nc.tensor.matmul(
                    ops_[:, j, m, :], ones128[:], bo4[0:1, j, :],
                    start=False, stop=True,
                )
        nc.scalar.activation(out_sb[:], ops_[:], AF.Sigmoid)
        nc.sync.dma_start(out_d[:, :, :, :], out_sb[:])

    def _exchange(g):
        a2a_in = dram.tile([B, EW[g]], F16, name=f"a2ain{g}", tag=f"a2ain{g}")
        a2a_out = dram.tile([B, EW[g]], F16, name=f"a2aout{g}", tag=f"a2aout{g}")
        nc.sync.dma_start(a2a_in[:, :], flat_sb[:, ECUM[g] : ECUM[g + 1]])
        tc_ctx = tc.tile_wait_until(EXCH_MS[g])
        tc_ctx.__enter__()
        nc.gpsimd.collective_compute(
            "AllToAll",
            mybir.AluOpType.bypass,
            replica_groups=[list(range(NCORES))],
            ins=[a2a_in.opt()],
            outs=[a2a_out.opt()],
        )
        # assemble group g's contiguous global param range for my 4 samples.
        # Last group goes through Pool (idle after its collective) so the ACT
        # queue never blocks behind the final exchange.
        eng = nc.gpsimd if g == NG - 1 else nc.scalar
        f4g = dram.tile([BPC, GBS[g]], F16, name=f"flat4g{g}", tag=f"flat4g{g}")
        eng.dma_start(
            f4g.rearrange("r (s c) -> r s c", c=EW[g]),
            a2a_out.rearrange("(s r) q -> r s q", r=BPC),
        )
        _extract_pieces(g, f4g, eng, nc.scalar if g == NG - 1 else None)
        # bias converts for params completed by this group
        if g == 0:
            nc.scalar.copy(binf[:], bin4[:])
            nc.scalar.copy(bhfs[0][:], bh4s[0][:])
        elif g == 1:
            nc.scalar.copy(bhfs[1][:], bh4s[1][:])
        elif g == NG - 1:
            nc.vector.tensor_copy(bhfs[2][:], bh4s[2][:])
        tc_ctx.__exit__(None, None, None)

    # ---- flat shard = h2 @ W3c.T + b3c, streamed in PSUM-sized subgroups;
    #      exchanges + CoordinateNet layers interleave into the stream.
    flat_sb = const.tile([B, S], F16, name="flat_sb", tag="flat_sb")

    import os as _os
    _sc = float(_os.environ.get("HINT_SCALE", "1.0"))
    EXCH_MS = [1.0 * _sc, 2.0 * _sc, 3.0 * _sc, 4.0 * _sc]
    stage_after = {len(SSG) - 1: [_input_layer, lambda: _hidden_layer(0),
                                  lambda: _hidden_layer(1),
                                  lambda: _hidden_layer(2), _output_layer]}

    next_g = 0
    for i in range(len(SSG)):
        w_s = SSG[i]
        chw = [CH] * (w_s // CH) + ([w_s % CH] if w_s % CH else [])
        nch = len(chw)
        fps = [
            psum.tile([B, chw[j]], FP, name=f"fps{i}_{j}", tag="ps")
            for j in range(nch)
        ]
        b3sb = b3p.tile([1, w_s], F16, name="b3sb", tag="b3sb")
        nc.sync.dma_start(b3sb[:], w3_d[H2 : H2 + 1, SCUM[i] : SCUM[i] + w_s])
        for kk in range(4):
            w3sb = w3p.tile([128, 2, w_s], F16, name="w3sb", tag="w3sb")
            nc.sync.dma_start(
                w3sb[:],
                w3_d[ts(kk, 256), SCUM[i] : SCUM[i] + w_s].rearrange(
                    "(t p) c -> p t c", p=128
                ),
            )
            for t in range(2):
                k = kk * 2 + t
                for j in range(nch):
                    nc.tensor.matmul(
                        fps[j][:], h2h[:, k, :],
                        w3sb[:, t, j * CH : j * CH + chw[j]],
                        start=(k == 0), stop=False,
                    )
        for j in range(nch):
            nc.tensor.matmul(
                fps[j][:], ones16[:],
                b3sb[:, j * CH : j * CH + chw[j]],
                start=False, stop=True,
            )
            nc.vector.tensor_copy(
                flat_sb[:, SCUM[i] + j * CH : SCUM[i] + j * CH + chw[j]],
                fps[j][:],
            )

        if i == len(SSG) - 1:
            # gate the CoordinateNet's input on the final drain: tss2 = 0*flat + ts
            nc.vector.scalar_tensor_tensor(
                tss2[:],
                flat_sb[0:1, 0 : BPC * NPTS].rearrange("a (j n) -> a j n", j=BPC),
                0.0,
                tssb[:],
                mybir.AluOpType.mult,
                mybir.AluOpType.add,
            )
        while next_g < NG and ECUM[next_g + 1] <= SCUM[i + 1]:
            _exchange(next_g)
            next_g += 1
        for fn in stage_after.get(i, []):
            fn()


_NC_CACHE = {}


def _get_module(repeat: int = 1):
    if repeat not in _NC_CACHE:
        _NC_CACHE[repeat] = _build_module(repeat)
    return _NC_CACHE[repeat]


# -------------------------------------------------------------- host wrapper
def _build_perm():
    perm = np.arange(P_TOTAL, dtype=np.int64)
    g = np.arange(HID * HID, dtype=np.int64).reshape(HID, HID)
    for a in WH_OFF:
        perm[a : a + HID * HID] = a + g.T.ravel()
    g2 = np.arange(4 * HID, dtype=np.int64).reshape(4, HID)
    perm[WO_OFF : WO_OFF + 4 * HID] = WO_OFF + g2.T.ravel()
    return perm


_PERM_CACHE = None
LAST_RESULTS = None


def prepare_in_maps(z, timestamps, W1, b1, W2, b2, W3, b3):
    global _PERM_CACHE
    z = np.asarray(z, np.float32)
    timestamps = np.asarray(timestamps, np.float32)
    W1 = np.asarray(W1, np.float32)
    b1 = np.asarray(b1, np.float32)
    W2 = np.asarray(W2, np.float32)
    b2 = np.asarray(b2, np.float32)
    W3 = np.asarray(W3, np.float32)
    b3 = np.asarray(b3, np.float32)

    if _PERM_CACHE is None:
        _PERM_CACHE = _build_perm()
    perm = _PERM_CACHE

    # [128, 32, 32] partition-major: zt[p, t, b] = z[b, t*128+p]
    zt = np.ascontiguousarray(
        z.T.reshape(LAT // 128, 128, B).transpose(1, 0, 2)
    ).astype(np.float16)
    w1t = np.ascontiguousarray(W1.T).astype(np.float16)
    w2t = np.ascontiguousarray(W2.T)
    Wp = W3[perm]
    bp = b3[perm]

    Wp_pad = np.zeros((P_PAD, H2), np.float16)
    Wp_pad[:P_TOTAL] = Wp.astype(np.float16)
    bp_pad = np.zeros((P_PAD,), np.float16)
    bp_pad[:P_TOTAL] = bp.astype(np.float16)

    in_maps = []
    for c in range(NCORES):
        w3_c = np.zeros((H2 + 1, S), np.float16)
        for g in range(NG):
            lo = GSTART[g] + c * EW[g]
            cs = slice(ECUM[g], ECUM[g + 1])
            w3_c[:H2, cs] = Wp_pad[lo : lo + EW[g]].T
            w3_c[H2, cs] = bp_pad[lo : lo + EW[g]]
        in_maps.append(
            {
                "zt": zt,
                "w1t": w1t,
                "b1": b1,
                "w2t": w2t,
                "b2": b2,
                "w3": w3_c,
                "tst": np.ascontiguousarray(
                    timestamps[c * BPC : (c + 1) * BPC, :, 0]
                ).astype(np.float16),
            }
        )
    return in_maps


def kernel(z, timestamps, W1, b1, W2, b2, W3, b3):
    global LAST_RESULTS
    in_maps = prepare_in_maps(z, timestamps, W1, b1, W2, b2, W3, b3)
    nc = _get_module(1)
    res = run_bass_kernel_spmd(nc, in_maps, core_ids=list(range(NCORES)))
    LAST_RESULTS = res
    outs = []
    for c in range(NCORES):
        o = np.asarray(res.results[c]["out"])  # [128, BPC, 4m, 4o]
        outs.append(o.transpose(1, 2, 0, 3).reshape(BPC, NPTS, 4))
    return np.concatenate(outs, axis=0).astype(np.float32, copy=False)
